# revision 5
# baseline (speedup 1.0000x reference)
"""Trainium2 Bass kernel for nn_Encoder_34943853920780 (gnn_message_passing).

Computation (see reference): build a degree-rank permuted +-1 hypervector
table, then for each unique undirected edge multiply the two endpoint rows
elementwise and sum over edges -> [D] output.

Strategy:
  Host (numpy, cheap index math):
    - in-degree, stable rank by degree, edge canonicalize + dedup
    - remap edge endpoints through the rank permutation -> row ids into w
    - shard edges across 8 cores; per core, compact the referenced rows
      into a private table (<= 2*cap+128 rows) so gather indices fit int16
  Device (8 NeuronCores, SPMD, Tile framework):
    - dma_gather (SWDGE) the two endpoint rows per edge (bf16, 2KB rows)
    - DVE: elementwise multiply + reduce over edges into a [128, D] f32
      accumulator (partition p holds edges == p mod 128)
    - DMA accumulator to DRAM
  Host: sum the per-core [128, D] partials -> [D].

All arithmetic is exact: products are +-1, f32 accumulation of <=100000
integer terms has no rounding, so the result matches the reference bit-for-bit.
"""
import sys

sys.path.insert(0, "/opt/trn_rl_repo")

import numpy as np
import ml_dtypes

import concourse.bacc as bacc
import concourse.mybir as mybir
import concourse.tile as tile
from concourse.bass_utils import run_bass_kernel_spmd

NCORES = 8
K = 512                     # edges per gather chunk (multiple of 128)
T = K // 128                # SBUF tiles per chunk
GBUFS = 9                   # gather pool chunk-slots in flight
WBUFS = 3                   # work pool slots
NQ = 4                      # SWDGE queues

# set by test harnesses: TRACE -> run with NTFF profiling; LAST_EXEC_NS holds
# the max-core exec time of the most recent kernel() call (None if untraced)
TRACE = False
LAST_EXEC_NS = None

_PROGRAM_CACHE = {}


def _build_program(D, cap, tbl_rows, nchunk):
    nc = bacc.Bacc("TRN2", num_swdge_queues=NQ)
    table = nc.dram_tensor("table", [tbl_rows, D], mybir.dt.bfloat16, kind="ExternalInput")
    idx = nc.dram_tensor("idx", [128, 2 * cap // 16], mybir.dt.int16, kind="ExternalInput")
    out = nc.dram_tensor("out", [128, D], mybir.dt.float32, kind="ExternalOutput")

    with tile.TileContext(nc) as tc:
        with (
            tc.tile_pool(name="persist", bufs=1) as pp,
            tc.tile_pool(name="gath", bufs=GBUFS) as gp,
            tc.tile_pool(name="work", bufs=WBUFS) as wp,
        ):
            idx_sb = pp.tile([128, 2 * cap // 16], mybir.dt.int16)
            acc_el = pp.tile([128, T, D], mybir.dt.bfloat16)  # per-slot sums, |v| <= nchunk
            acc = pp.tile([128, D], mybir.dt.float32)
            # chunk-0 indices land first so gathers start ~10us earlier
            H = 2 * K // 16
            nc.sync.dma_start(idx_sb[:, :H], idx[:, :H])
            nc.sync.dma_start(idx_sb[:, H:], idx[:, H:])
            nc.vector.memset(acc_el[:, :, :], 0.0)
            for c in range(nchunk):
                # one merged gather per chunk: K a-rows (tiles 0..T) + K b-rows (tiles T..2T)
                cols = slice(c * (2 * K // 16), (c + 1) * (2 * K // 16))
                tab = gp.tile([128, 2 * T, D], mybir.dt.bfloat16)
                nc.gpsimd.dma_gather(tab[:, :, :], table[:, :], idx_sb[:, cols],
                                     2 * K, 2 * K, D, queue_num=c % NQ)
                prod = wp.tile([128, T, D], mybir.dt.bfloat16)
                nc.vector.tensor_tensor(prod[:, :, :], tab[:, :T, :], tab[:, T:, :],
                                        mybir.AluOpType.mult)
                nc.vector.tensor_tensor(acc_el[:, :, :], acc_el[:, :, :], prod[:, :, :],
                                        mybir.AluOpType.add)
            nc.vector.tensor_reduce(acc[:, :], acc_el[:, :, :].transpose([0, 2, 1]),
                                    mybir.AxisListType.X, mybir.AluOpType.add)
            nc.sync.dma_start(out[:, :], acc[:, :])
    nc.compile()
    return nc


def _get_program(D, cap, tbl_rows, nchunk):
    key = (D, cap, tbl_rows, nchunk)
    if key not in _PROGRAM_CACHE:
        _PROGRAM_CACHE[key] = _build_program(D, cap, tbl_rows, nchunk)
    return _PROGRAM_CACHE[key]


def _wrap_idxs(idx_a, idx_b):
    """Merge a/b per chunk (a slots then b slots), SWDGE wrap: slot k of a
    2K-chunk -> [k%16, chunk_base + k//16], tiled x8 across 128 partitions."""
    cols = []
    for c in range(0, idx_a.size, K):
        chunk = np.concatenate([idx_a[c:c + K], idx_b[c:c + K]])
        cols.append(chunk.reshape(2 * K // 16, 16).T)
    w16 = np.concatenate(cols, axis=1)
    return np.ascontiguousarray(np.tile(w16, (8, 1))).astype(np.int16)


def kernel(node_ids_weight, edge_index, num_nodes):
    global LAST_EXEC_NS
    w = np.asarray(node_ids_weight)
    ei = np.asarray(edge_index)
    N = int(num_nodes)
    D = w.shape[1]

    # ---- host index preprocessing (mirrors reference semantics exactly) ----
    col = ei[1].astype(np.int64)
    degree = np.bincount(col, minlength=N)[:N]
    order = np.argsort(degree, kind="stable")       # == stable argsort of degree/N
    rank = np.empty(N, np.int64)
    rank[order] = np.arange(N)                      # node_id_hvs[j] == w[rank[j]]

    lo = np.minimum(ei[0], ei[1]).astype(np.int64)
    hi = np.maximum(ei[0], ei[1]).astype(np.int64)
    uniq = np.unique(lo * N + hi)                   # unique undirected edges
    ra = rank[uniq // N]                            # row ids into w, [U]
    rb = rank[uniq % N]
    U = ra.size

    per = -(-U // NCORES)                           # edges per core
    cap = -(-per // K) * K                          # padded to chunk multiple
    nchunk = cap // K
    tbl_rows = 2 * cap + 128                        # worst-case distinct rows + zero row
    assert tbl_rows <= 32768, "indices must fit int16"

    w_bf = w[:N].astype(ml_dtypes.bfloat16)

    in_maps = []
    for i in range(NCORES):
        sl = slice(i * per, min((i + 1) * per, U))
        a_i, b_i = ra[sl], rb[sl]
        n_i = a_i.size
        u, inv = np.unique(np.concatenate([a_i, b_i]), return_inverse=True)
        idx_a = np.zeros(cap, np.int16)
        idx_b = np.zeros(cap, np.int16)
        idx_a[:n_i] = inv[:n_i] + 1                 # local row 0 is the zero row
        idx_b[:n_i] = inv[n_i:] + 1
        table = np.empty((tbl_rows, D), ml_dtypes.bfloat16)
        table[0] = 0
        table[1:1 + u.size] = w_bf[u]
        in_maps.append({
            "table": table,
            "idx": _wrap_idxs(idx_a, idx_b),
        })

    nc = _get_program(D, cap, tbl_rows, nchunk)
    res = run_bass_kernel_spmd(nc, in_maps, list(range(NCORES)), trace=TRACE)
    LAST_EXEC_NS = res.exec_time_ns

    total = np.zeros(D, np.float32)
    for i in range(NCORES):
        total += res.results[i]["out"].sum(axis=0, dtype=np.float32)
    return total.astype(np.float32)


# revision 6
# speedup vs baseline: 1.1875x; 1.1875x over previous
"""Trainium2 Bass kernel for nn_Encoder_34943853920780 (gnn_message_passing).

Computation (see reference): build a degree-rank permuted +-1 hypervector
table, then for each unique undirected edge multiply the two endpoint rows
elementwise and sum over edges -> [D] output.

Strategy:
  Host (numpy, cheap index math):
    - in-degree, stable rank by degree, edge canonicalize + dedup
    - remap edge endpoints through the rank permutation -> row ids into w
    - shard edges across 8 cores; per core, compact the referenced rows
      into a private table (<= 2*cap+128 rows) so gather indices fit int16
  Device (8 NeuronCores, SPMD, Tile framework):
    - dma_gather (SWDGE) the two endpoint rows per edge (bf16, 2KB rows)
    - DVE: elementwise multiply + reduce over edges into a [128, D] f32
      accumulator (partition p holds edges == p mod 128)
    - DMA accumulator to DRAM
  Host: sum the per-core [128, D] partials -> [D].

All arithmetic is exact: products are +-1, f32 accumulation of <=100000
integer terms has no rounding, so the result matches the reference bit-for-bit.
"""
import sys

sys.path.insert(0, "/opt/trn_rl_repo")

import numpy as np
import ml_dtypes

import concourse.bacc as bacc
import concourse.mybir as mybir
import concourse.tile as tile
from concourse.bass_utils import run_bass_kernel_spmd

NCORES = 8
K = 512                     # edges per gather chunk (multiple of 128)
T = K // 128                # SBUF tiles per chunk
GBUFS = 9                   # gather pool chunk-slots in flight
WBUFS = 3                   # work pool slots
NQ = 4                      # SWDGE queues

# set by test harnesses: TRACE -> run with NTFF profiling; LAST_EXEC_NS holds
# the max-core exec time of the most recent kernel() call (None if untraced)
TRACE = False
LAST_EXEC_NS = None

_PROGRAM_CACHE = {}


def _build_program(D, cap, tbl_rows, nchunk):
    nc = bacc.Bacc("TRN2", num_swdge_queues=NQ)
    table = nc.dram_tensor("table", [tbl_rows, D], mybir.dt.bfloat16, kind="ExternalInput")
    idx = nc.dram_tensor("idx", [128, 2 * cap // 16], mybir.dt.int16, kind="ExternalInput")
    out = nc.dram_tensor("out", [128, D], mybir.dt.float32, kind="ExternalOutput")

    with tile.TileContext(nc) as tc:
        with (
            tc.tile_pool(name="persist", bufs=1) as pp,
            tc.tile_pool(name="gath", bufs=GBUFS) as gp,
            tc.tile_pool(name="work", bufs=WBUFS) as wp,
        ):
            idx_sb = pp.tile([128, 2 * cap // 16], mybir.dt.int16)
            acc_el = pp.tile([128, T, D], mybir.dt.bfloat16)  # per-slot sums, |v| <= nchunk
            acc = pp.tile([128, D], mybir.dt.float32)
            nc.sync.dma_start(idx_sb[:, :], idx[:, :])
            nc.vector.memset(acc_el[:, :, :], 0.0)
            for c in range(nchunk):
                # one merged gather per chunk: K a-rows (tiles 0..T) + K b-rows (tiles T..2T)
                cols = slice(c * (2 * K // 16), (c + 1) * (2 * K // 16))
                tab = gp.tile([128, 2 * T, D], mybir.dt.bfloat16)
                nc.gpsimd.dma_gather(tab[:, :, :], table[:, :], idx_sb[:, cols],
                                     2 * K, 2 * K, D, queue_num=c % NQ)
                prod = wp.tile([128, T, D], mybir.dt.bfloat16)
                nc.vector.tensor_tensor(prod[:, :, :], tab[:, :T, :], tab[:, T:, :],
                                        mybir.AluOpType.mult)
                nc.vector.tensor_tensor(acc_el[:, :, :], acc_el[:, :, :], prod[:, :, :],
                                        mybir.AluOpType.add)
            nc.vector.tensor_reduce(acc[:, :], acc_el[:, :, :].transpose([0, 2, 1]),
                                    mybir.AxisListType.X, mybir.AluOpType.add)
            nc.sync.dma_start(out[:, :], acc[:, :])
    nc.compile()
    return nc


def _get_program(D, cap, tbl_rows, nchunk):
    key = (D, cap, tbl_rows, nchunk)
    if key not in _PROGRAM_CACHE:
        _PROGRAM_CACHE[key] = _build_program(D, cap, tbl_rows, nchunk)
    return _PROGRAM_CACHE[key]


def _wrap_idxs(idx_a, idx_b):
    """Merge a/b per chunk (a slots then b slots), SWDGE wrap: slot k of a
    2K-chunk -> [k%16, chunk_base + k//16], tiled x8 across 128 partitions."""
    cols = []
    for c in range(0, idx_a.size, K):
        chunk = np.concatenate([idx_a[c:c + K], idx_b[c:c + K]])
        cols.append(chunk.reshape(2 * K // 16, 16).T)
    w16 = np.concatenate(cols, axis=1)
    return np.ascontiguousarray(np.tile(w16, (8, 1))).astype(np.int16)


def kernel(node_ids_weight, edge_index, num_nodes):
    global LAST_EXEC_NS
    w = np.asarray(node_ids_weight)
    ei = np.asarray(edge_index)
    N = int(num_nodes)
    D = w.shape[1]

    # ---- host index preprocessing (mirrors reference semantics exactly) ----
    col = ei[1].astype(np.int64)
    degree = np.bincount(col, minlength=N)[:N]
    order = np.argsort(degree, kind="stable")       # == stable argsort of degree/N
    rank = np.empty(N, np.int64)
    rank[order] = np.arange(N)                      # node_id_hvs[j] == w[rank[j]]

    lo = np.minimum(ei[0], ei[1]).astype(np.int64)
    hi = np.maximum(ei[0], ei[1]).astype(np.int64)
    uniq = np.unique(lo * N + hi)                   # unique undirected edges
    ra = rank[uniq // N]                            # row ids into w, [U]
    rb = rank[uniq % N]
    U = ra.size

    per = -(-U // NCORES)                           # edges per core
    cap = -(-per // K) * K                          # padded to chunk multiple
    nchunk = cap // K
    tbl_rows = 2 * cap + 128                        # worst-case distinct rows + zero row
    assert tbl_rows <= 32768, "indices must fit int16"

    w_bf = w[:N].astype(ml_dtypes.bfloat16)

    in_maps = []
    for i in range(NCORES):
        sl = slice(i * per, min((i + 1) * per, U))
        a_i, b_i = ra[sl], rb[sl]
        n_i = a_i.size
        u, inv = np.unique(np.concatenate([a_i, b_i]), return_inverse=True)
        idx_a = np.zeros(cap, np.int16)
        idx_b = np.zeros(cap, np.int16)
        idx_a[:n_i] = inv[:n_i] + 1                 # local row 0 is the zero row
        idx_b[:n_i] = inv[n_i:] + 1
        table = np.empty((tbl_rows, D), ml_dtypes.bfloat16)
        table[0] = 0
        table[1:1 + u.size] = w_bf[u]
        in_maps.append({
            "table": table,
            "idx": _wrap_idxs(idx_a, idx_b),
        })

    nc = _get_program(D, cap, tbl_rows, nchunk)
    res = run_bass_kernel_spmd(nc, in_maps, list(range(NCORES)), trace=TRACE)
    LAST_EXEC_NS = res.exec_time_ns

    total = np.zeros(D, np.float32)
    for i in range(NCORES):
        total += res.results[i]["out"].sum(axis=0, dtype=np.float32)
    return total.astype(np.float32)
